# revision 67
# baseline (speedup 1.0000x reference)
"""Trainium2 Bass kernel for MoE-LoRA GQA attention (nn_Attention_57389353009692).

Strategy (8 NeuronCores, one SPMD launch), v2:
  - Tensor-parallel over heads: core c owns q-heads 4c..4c+3 and kv-head c.
  - Phase A (per 512-token block): packed QKV projections (q0|q1|kv plus
    LoRA-A/router packs, 5 PSUM accumulators), router softmax entirely
    on-chip (exp -> ones-matmul sum -> reciprocal -> selector-matmul
    partition broadcast; no transposes, no DRAM roundtrip), fp16 RoPE on
    full 128-partition tiles, head rearrange via small SBUF DMAs.
  - Phase C: flash-style attention at (128 key x 256 query) granularity.
    All 4 q-heads share one kv head (GQA), so one score matmul covers two
    heads (moving = 2x256 queries). Scores in fp32 PSUM (ping-ponged
    2-bank tiles), one fused exp per half-block with a constant bias
    2^-8 folded in (cancels in normalization), mask-add only on partial
    diagonal half-blocks using deduped mask patterns. Unnormalized AV sums
    + denominators accumulate in one 4-bank PSUM tile per 512-query block.
  - Two AllToAlls (tokens 0-1023 after qb1, 1024-2047 after qb3) reshard
    head-major -> token-major; the first overlaps with qb2/qb3 attention.
  - Phase D per 128-token half: normalize by denominators (selector-matmul
    broadcast), o-projection against the full wo (prefetched during phase
    C) + o-LoRA, direct PSUM->DRAM output.

Numerics: fp16 operands for all matmuls (accumulation fp32 in PSUM),
softmaxes in fp32 without max-subtraction (scores are O(1); the mask is
clamped to -60000 so fp16/exp underflow to exactly 0). Scale 1/sqrt(64)
folded into wq and the q-LoRA B on host.
"""

import os
import sys

for _p in ("/opt/trn_rl_repo", "/root/.axon_site/_ro/trn_rl_repo"):
    if _p not in sys.path:
        sys.path.insert(0, _p)

KDBG = os.environ.get("KDBG", "")

import numpy as np
import ml_dtypes

import concourse.bass as bass
import concourse.tile as tile
from concourse import bacc, mybir
from concourse.masks import make_identity
from concourse.alu_op_type import AluOpType

F32 = mybir.dt.float32
FP16 = mybir.dt.float16
AF = mybir.ActivationFunctionType
FP16NP = np.float16

B, S, D = 1, 2048, 2048
H, KVH, HD = 32, 8, 64
R, E = 8, 8
SCALING = 32.0 / 8.0
NCORES = 8
QH = H // NCORES          # 4 q heads per core
QF = QH * HD              # 256 q feats per core
NKT = S // 128            # 16 key tiles
NQB = S // 512            # 4 query blocks
NIF = D // 128            # 16 contraction tiles
TD = S // NCORES // 2     # 128 tokens per core per A2A half

MASK_NEG = -60000.0
EXP_BIAS = -5.545177444479562  # -8*ln2: pr scaled 2^-8, cancels in norm

M_SKIP, M_FREE = -2, -1  # cls >= 0 -> mask pattern index


def _build_perm():
    idx_q = np.zeros(QF, dtype=np.int64)
    for f in range(QF):
        blk, w = divmod(f, 128)
        h, j = divmod(w, 32)
        idx_q[f] = 64 * h + 2 * j + blk
    idx_k = np.zeros(HD, dtype=np.int64)
    for w in range(HD):
        idx_k[w] = 2 * w if w < 32 else 2 * (w - 32) + 1
    return idx_q, idx_k


IDX_Q, IDX_K = _build_perm()


def _fp16(x):
    return np.ascontiguousarray(np.asarray(x, dtype=np.float32)).astype(FP16NP)


def _f32(x):
    return np.ascontiguousarray(x, dtype=np.float32)


def _a_pack(A):
    """[E,R,D] -> [D, 64] columns ordered r*8+e."""
    return np.transpose(A, (1, 0, 2)).reshape(E * R, -1).T


def _b_flat(Bw, scale):
    """[E, OF, R] -> [64, OF] with row r*8+e."""
    return np.transpose(Bw, (2, 0, 1)).reshape(E * R, -1) * scale


def _sw(W):
    """[D, F] -> [128, NIF*F] pre-swizzled to the SBUF [p, n, f] layout."""
    F = W.shape[1]
    return W.reshape(NIF, 128, F).transpose(1, 0, 2).reshape(128, NIF * F)


def classify_mask(maskT):
    """maskT: [S(k), S(q)] clamped fp32.

    Returns cls[NKT, NQB, 2] with M_SKIP / M_FREE / pattern-index, and the
    deduped pattern list [[128, 256] fp32, ...].
    """
    cls = np.full((NKT, NQB, 2), M_SKIP, dtype=np.int64)
    patterns = []
    keys = {}
    for qb in range(NQB):
        for kt in range(NKT):
            rows = maskT[kt * 128:(kt + 1) * 128]
            for hf in range(2):
                blk = rows[:, qb * 512 + hf * 256: qb * 512 + hf * 256 + 256]
                if np.all(blk <= MASK_NEG * 0.5):
                    cls[kt, qb, hf] = M_SKIP
                elif np.all(blk == 0.0):
                    cls[kt, qb, hf] = M_FREE
                else:
                    kb = blk.astype(np.float32).tobytes()
                    if kb not in keys:
                        keys[kb] = len(patterns)
                        patterns.append(blk.astype(np.float32))
                    cls[kt, qb, hf] = keys[kb]
    return cls, patterns


def build(cls, n_pat):
    nc = bacc.Bacc(None, target_bir_lowering=False)

    xT = nc.declare_dram_parameter("xT", [D, S], FP16, isOutput=False)
    # weight packs pre-swizzled on host to [128, NIF*F] (SBUF layout)
    wqT = nc.declare_dram_parameter("wqT", [128, NIF * QF], FP16,
                                    isOutput=False)
    wkvT = nc.declare_dram_parameter("wkvT", [128, NIF * 128], FP16,
                                     isOutput=False)
    aqk = nc.declare_dram_parameter("aqk", [128, NIF * 128], FP16,
                                    isOutput=False)
    avr = nc.declare_dram_parameter("avr", [128, NIF * 88], FP16,
                                    isOutput=False)
    aob = nc.declare_dram_parameter("aob", [128, NIF * 72], FP16,
                                    isOutput=False)
    bq = nc.declare_dram_parameter("bq", [64, QF], FP16, isOutput=False)
    bkv = nc.declare_dram_parameter("bkv", [128, 64], FP16, isOutput=False)
    bv = nc.declare_dram_parameter("bv", [64, 64], FP16, isOutput=False)
    bo = nc.declare_dram_parameter("bo", [64, D], FP16, isOutput=False)
    woT = nc.declare_dram_parameter("woT", [128, NIF * D], FP16,
                                    isOutput=False)
    cs4 = nc.declare_dram_parameter("cs4", [128, S], FP16, isOutput=False)
    sn4 = nc.declare_dram_parameter("sn4", [128, S], FP16, isOutput=False)
    sels = nc.declare_dram_parameter("sels", [24, 516], FP16, isOutput=False)
    selk = nc.declare_dram_parameter("selk", [32, NIF * 128], FP16,
                                     isOutput=False)
    masku = nc.declare_dram_parameter("masku", [128, max(n_pat, 1) * 256],
                                      FP16, isOutput=False)
    y = nc.declare_dram_parameter("y", [2 * TD, D], F32, isOutput=True)

    cc_in = [nc.dram_tensor(f"cc{i}_in", [NCORES, QF + QH, TD], FP16)
             for i in range(2)]
    cc_out = [nc.dram_tensor(f"cc{i}_out", [NCORES, QF + QH, TD], FP16)
              for i in range(2)]

    with tile.TileContext(nc) as tc:
        _emit(nc, tc, locals(), cls, n_pat)
    nc.finalize()
    return nc


def _emit(nc, tc, t, cls, n_pat):
    import contextlib
    ctx = contextlib.ExitStack()
    with ctx:
        persist = ctx.enter_context(tc.tile_pool(name="persist", bufs=1))

        # ---- persistent tiles (allocated up front; DMAs issued in
        # priority order below: descriptors fan out over all 16 DMA
        # engines, so aggregate HBM bw ~= issue order = arrival order) ----
        ENGS = (nc.gpsimd, nc.sync, nc.scalar)
        xv = t["xT"].rearrange("(n p) s -> p n s", p=128)
        aqk_q, wqT_q = [], []
        for j in range(4):
            aqk_q.append(persist.tile([128, 4, 128], FP16,
                                      name="aqk%d" % j))
            wqT_q.append(persist.tile([128, 4, QF], FP16, name="wqT%d" % j))
        wkvT_sb = persist.tile([128, NIF, 128], FP16)
        avr_sb = persist.tile([128, NIF, 88], FP16)
        aob_sb = persist.tile([128, NIF, 72], FP16)
        cs4_sb = persist.tile([128, S], FP16)
        sn4_sb = persist.tile([128, S], FP16)
        bq_sb = persist.tile([64, QF], FP16)
        bkv_sb = persist.tile([128, 64], FP16)
        bv_sb = persist.tile([64, 64], FP16)
        bo_sb = persist.tile([64, D], FP16)
        sels_sb = persist.tile([24, 516], FP16)
        selk_sb = persist.tile([32, NIF, 128], FP16)
        masku_sb = persist.tile([128, max(n_pat, 1), 256], FP16)
        ident_h = persist.tile([64, 64], FP16)
        ebias = persist.tile([128, 1], F32)
        qh_sb = persist.tile([64, QH, S], FP16)
        kh_sb = persist.tile([64, S], FP16)
        vT_sb = persist.tile([64, S], FP16)
        vtok = persist.tile([128, NKT, 65], FP16)
        woT_sb = persist.tile([128, NIF, D], FP16)  # DMAs issued after A

        sel_qk = sels_sb[:, 0:128]
        sel_v = sels_sb[:, 128:192]
        ones_blk = sels_sb[:, 192:216]   # [24, 24] block-diag ones
        sel_o = sels_sb[0:8, 216:280]
        ones8x8 = sels_sb[0:8, 280:288]  # [8, 8] all ones

        pA_ctx = tc.tile_pool(name="pA", bufs=1)
        pA = pA_ctx.__enter__()
        xqs0 = [pA.tile([128, 4, 512], FP16, name="xq%d" % qtr,
                        tag="xq%d" % qtr, bufs=2) for qtr in range(4)]

        # ---- DMA issue, priority order ----
        nc.gpsimd.dma_start(
            out=aqk_q[0], in_=t["aqk"][:, 0:512]
            .rearrange("p (n f) -> p n f", n=4))
        nc.sync.dma_start(out=xqs0[0], in_=xv[:, 0:4, 0:512])
        nc.scalar.dma_start(
            out=wqT_q[0], in_=t["wqT"][:, 0:1024]
            .rearrange("p (n f) -> p n f", n=4))
        for j in range(1, 4):
            nc.sync.dma_start(out=xqs0[j],
                              in_=xv[:, 4 * j:4 * j + 4, 0:512])
            nc.gpsimd.dma_start(
                out=aqk_q[j], in_=t["aqk"][:, 512 * j:512 * j + 512]
                .rearrange("p (n f) -> p n f", n=4))
            nc.scalar.dma_start(
                out=wqT_q[j], in_=t["wqT"][:, 1024 * j:1024 * j + 1024]
                .rearrange("p (n f) -> p n f", n=4))
        for nm, tile_, w in (("avr", avr_sb, 88), ("wkvT", wkvT_sb, 128)):
            for j in range(2):
                nsl = slice(8 * j, 8 * j + 8)
                ENGS[j].dma_start(
                    out=tile_[:, nsl, :],
                    in_=t[nm][:, 8 * w * j:8 * w * j + 8 * w]
                    .rearrange("p (n f) -> p n f", n=8))
        for j in range(2):
            hs = slice(1024 * j, 1024 * j + 1024)
            nc.sync.dma_start(out=cs4_sb[:, hs], in_=t["cs4"][:, hs])
            ENGS[j].dma_start(out=sn4_sb[:, hs], in_=t["sn4"][:, hs])
        for j in range(2):
            nsl = slice(8 * j, 8 * j + 8)
            ENGS[j].dma_start(
                out=aob_sb[:, nsl, :],
                in_=t["aob"][:, 576 * j:576 * j + 576]
                .rearrange("p (n f) -> p n f", n=8))
        nc.gpsimd.dma_start(out=bq_sb, in_=t["bq"][:])
        nc.gpsimd.dma_start(out=bkv_sb, in_=t["bkv"][:])
        nc.gpsimd.dma_start(out=bv_sb, in_=t["bv"][:])
        nc.gpsimd.dma_start(out=bo_sb, in_=t["bo"][:])
        nc.gpsimd.dma_start(out=sels_sb, in_=t["sels"][:])
        nc.gpsimd.dma_start(
            out=selk_sb, in_=t["selk"].rearrange("h (n f) -> h n f", f=128))
        nc.gpsimd.dma_start(
            out=masku_sb,
            in_=t["masku"].rearrange("p (u f) -> p u f", f=256))
        make_identity(nc, ident_h)
        nc.vector.memset(ebias, EXP_BIAS)
        nc.vector.memset(vtok[:, :, 64:65], 1.0)

        # ================= Phase A =================
        with nc.named_scope("phaseA"), \
                tc.tile_pool(name="psA", bufs=1, space="PSUM") as ps:
            for tb in range(4):
                tsl = slice(tb * 512, (tb + 1) * 512)
                # quarter tiles so the first k-chunks unblock the PE early
                if tb == 0:
                    xqs = xqs0
                else:
                    xqs = []
                    for qtr in range(4):
                        xqt = pA.tile([128, 4, 512], FP16,
                                      name="xq%d" % qtr,
                                      tag="xq%d" % qtr, bufs=2)
                        nc.sync.dma_start(
                            out=xqt, in_=xv[:, 4 * qtr:4 * qtr + 4, tsl])
                        xqs.append(xqt)

                def xq_(k):
                    return xqs[k // 4][:, k % 4, :]

                la = ps.tile([128, 512], F32, name="la", tag="b_la")
                lv = ps.tile([88, 512], F32, name="lv", tag="b_lv")
                q0 = ps.tile([128, 512], F32, name="q0", tag="b_q0")
                q1 = ps.tile([128, 512], F32, name="q1", tag="b_q1")
                kv = ps.tile([128, 512], F32, name="kv", tag="b_kv")
                lsc = ps.tile([128, 512], F32, name="lsc", tag="b_lsc")
                rwx = ps.tile([128, 512], F32, name="rwx", tag="b_rwx")
                vps = ps.tile([128, 2, 64], FP16, name="vps", tag="b_vps")

                for k in range(NIF):
                    st, sp = k == 0, k == NIF - 1
                    nc.tensor.matmul(la, aqk_q[k // 4][:, k % 4, :], xq_(k),
                                     start=st, stop=sp)
                for k in range(NIF):
                    nc.tensor.matmul(q0, wqT_q[k // 4][:, k % 4, 0:128], xq_(k),
                                     start=(k == 0), stop=False)
                for k in range(NIF):
                    st, sp = k == 0, k == NIF - 1
                    nc.tensor.matmul(lv, avr_sb[:, k, :], xq_(k),
                                     start=st, stop=sp)
                # router softmax (fp32, no max-subtract; logits lv[64:88])
                ex_h = pA.tile([24, 512], FP16, name="ex_h", tag="ex_h",
                               bufs=2)
                nc.scalar.activation(ex_h, lv[64:88, :], AF.Exp)
                la_sb = pA.tile([128, 512], FP16, name="la_sb", tag="la_sb",
                                bufs=2)
                nc.vector.tensor_copy(la_sb, la)

                for k in range(NIF):
                    nc.tensor.matmul(q1, wqT_q[k // 4][:, k % 4, 128:256], xq_(k),
                                     start=(k == 0), stop=False)
                # per-group softmax sums at 24 partitions -> lsc rows 0:24
                nc.tensor.matmul(lsc[0:24, :], ones_blk, ex_h,
                                 start=True, stop=True)
                rec24 = pA.tile([24, 512], FP16, name="rec24", tag="rec24",
                                bufs=2)
                with nc.allow_low_precision(reason="router softmax recip"):
                    nc.vector.reciprocal(rec24, lsc[0:24, :])
                rw_n = pA.tile([24, 512], FP16, name="rw_n", tag="rw_n",
                               bufs=2)
                nc.vector.tensor_tensor(rw_n, ex_h, rec24, AluOpType.mult)

                for k in range(NIF):
                    nc.tensor.matmul(kv, wkvT_sb[:, k, :], xq_(k),
                                     start=(k == 0), stop=False)
                nc.tensor.matmul(rwx, sel_qk, rw_n, start=True, stop=True)
                hpqk = pA.tile([128, 512], FP16, name="hpqk", tag="hpqk",
                               bufs=2)
                nc.vector.tensor_tensor(hpqk, la_sb, rwx, AluOpType.mult)
                # v-lora combine (reuses the lsc bank after rec24's read)
                nc.tensor.matmul(lsc[0:64, :], sel_v, rw_n,
                                 start=True, stop=True)
                lv_sb = pA.tile([64, 512], FP16, name="lv_sb", tag="lv_sb",
                                bufs=2)
                nc.vector.tensor_copy(lv_sb, lv[0:64, :])
                hpv = pA.tile([64, 512], FP16, name="hpv", tag="hpv", bufs=2)
                nc.vector.tensor_tensor(hpv, lv_sb, lsc[0:64, :],
                                        AluOpType.mult)

                # LoRA-B accumulations
                nc.tensor.matmul(q0, bq_sb[:, 0:128], hpqk[0:64, :],
                                 start=False, stop=True)
                nc.tensor.matmul(q1, bq_sb[:, 128:256], hpqk[0:64, :],
                                 start=False, stop=True)
                nc.tensor.matmul(kv[0:64, :], bkv_sb[64:128, :],
                                 hpqk[64:128, :], start=False, stop=True)
                nc.tensor.matmul(kv[64:128, :], bv_sb, hpv,
                                 start=False, stop=True)

                # ---- RoPE (fp16) ----
                q0c = pA.tile([128, 512], FP16, name="q0c", tag="q0c", bufs=2)
                nc.vector.tensor_copy(q0c, q0)
                q1c = pA.tile([128, 512], FP16, name="q1c", tag="q1c", bufs=2)
                nc.scalar.activation(q1c, q1, AF.Copy)
                csl = cs4_sb[:, tsl]
                snl = sn4_sb[:, tsl]
                t1 = pA.tile([128, 512], FP16, name="t1", tag="t1", bufs=2)
                t2 = pA.tile([128, 512], FP16, name="t2", tag="t2", bufs=2)
                qre = pA.tile([128, 512], FP16, name="qre", tag="qre", bufs=2)
                qro = pA.tile([128, 512], FP16, name="qro", tag="qro", bufs=2)
                nc.vector.tensor_tensor(t1, q0c, csl, AluOpType.mult)
                nc.vector.tensor_tensor(t2, q1c, snl, AluOpType.mult)
                nc.vector.tensor_tensor(qre, t1, t2, AluOpType.subtract)
                nc.vector.tensor_tensor(t1, q0c, snl, AluOpType.mult)
                nc.vector.tensor_tensor(t2, q1c, csl, AluOpType.mult)
                nc.vector.tensor_tensor(qro, t1, t2, AluOpType.add)

                kc2 = pA.tile([32, 2, 512], FP16, name="kc2", tag="kc2",
                              bufs=2)
                nc.vector.tensor_copy(kc2[:, 0, :], kv[0:32, :])
                nc.vector.tensor_copy(kc2[:, 1, :], kv[32:64, :])
                nc.scalar.activation(vT_sb[:, tsl], kv[64:128, :], AF.Copy)
                csl32 = cs4_sb[0:32, tsl]
                snl32 = sn4_sb[0:32, tsl]
                tk1 = pA.tile([32, 512], FP16, name="tk1", tag="tk1", bufs=2)
                tk2 = pA.tile([32, 512], FP16, name="tk2", tag="tk2", bufs=2)
                kho = pA.tile([32, 512], FP16, name="kho", tag="kho", bufs=2)
                nc.vector.tensor_tensor(tk1, kc2[:, 0, :], csl32,
                                        AluOpType.mult)
                nc.vector.tensor_tensor(tk2, kc2[:, 1, :], snl32,
                                        AluOpType.mult)
                nc.vector.tensor_tensor(kh_sb[0:32, tsl], tk1, tk2,
                                        AluOpType.subtract)
                nc.vector.tensor_tensor(tk1, kc2[:, 0, :], snl32,
                                        AluOpType.mult)
                nc.vector.tensor_tensor(tk2, kc2[:, 1, :], csl32,
                                        AluOpType.mult)
                nc.vector.tensor_tensor(kho, tk1, tk2, AluOpType.add)
                nc.vector.tensor_copy(kh_sb[32:64, tsl], kho)

                for h in range(QH):
                    nc.sync.dma_start(
                        out=qh_sb[0:32, h, tsl],
                        in_=qre[32 * h:32 * h + 32, :])
                    nc.sync.dma_start(
                        out=qh_sb[32:64, h, tsl],
                        in_=qro[32 * h:32 * h + 32, :])

                # token-major v
                for j in range(4):
                    kt = 4 * tb + j
                    nc.tensor.transpose(
                        vps[:, j % 2, :], vT_sb[:, 128 * kt:128 * kt + 128],
                        ident_h)
                    nc.vector.tensor_copy(vtok[:, kt, 0:64], vps[:, j % 2, :])

        pA_ctx.__exit__(None, None, None)

        # prefetch wo during attention: 8 chunks of 1 MB across queues
        wov = t["woT"].rearrange("p (n f) -> p n f", n=NIF)
        for j in range(8):
            nsl = slice(2 * j, 2 * j + 2)
            ENGS[j % 3].dma_start(out=woT_sb[:, nsl, :], in_=wov[:, nsl, :])

        if KDBG == "qkv":
            with tc.tile_pool(name="pX", bufs=1) as pX:
                for r, src in enumerate((qh_sb[:, 0, :], kh_sb, vT_sb,
                                         qh_sb[:, 1, :])):
                    ytd = pX.tile([64, S], F32, name="ytd", tag="ytd", bufs=2)
                    nc.vector.tensor_copy(ytd, src)
                    nc.sync.dma_start(out=t["y"][64 * r:64 * r + 64, :],
                                      in_=ytd)
            return

        # ================= Phase C =================
        sc_i = 0
        with nc.named_scope("phaseC"), \
                tc.tile_pool(name="pC", bufs=1) as pC, \
                tc.tile_pool(name="psC", bufs=1, space="PSUM") as ps:
            for qb in range(NQB):
                outp = ps.tile([65, 2, 2, 2, 256], F32, name="outp",
                               tag="outp")
                first = {}
                last = {}
                for hf in range(2):
                    act = [kt for kt in range(NKT)
                           if cls[kt, qb, hf] != M_SKIP]
                    first[hf], last[hf] = act[0], act[-1]
                for kt in range(4 * qb + 4):
                    ksl = slice(128 * kt, 128 * kt + 128)
                    for hf in range(2):
                        c = cls[kt, qb, hf]
                        if c == M_SKIP:
                            continue
                        qsl = slice(512 * qb + 256 * hf,
                                    512 * qb + 256 * hf + 256)
                        sc = ps.tile([128, 2, 2, 256], F32, name="sc",
                                     tag="b_sc%d" % (sc_i % 2))
                        sc_i += 1
                        for p in range(2):
                            nc.tensor.matmul(
                                sc[:, p, :, :], kh_sb[:, ksl],
                                qh_sb[:, 2 * p:2 * p + 2, qsl],
                                start=True, stop=True)
                        if c >= 0:
                            mt = masku_sb[:, c, :]
                            nc.vector.tensor_tensor(
                                sc, sc,
                                mt.unsqueeze(1).unsqueeze(1)
                                .broadcast_to([128, 2, 2, 256]),
                                AluOpType.add)
                        pr = pC.tile([128, 2, 2, 256], FP16, name="pr",
                                     tag="pr", bufs=3)
                        nc.scalar.activation(pr, sc, AF.Exp, bias=ebias)
                        for p in range(2):
                            nc.tensor.matmul(
                                outp[:, p, hf, :, :], vtok[:, kt, :],
                                pr[:, p, :, :],
                                start=(kt == first[hf]),
                                stop=(kt == last[hf]))
                on65 = pC.tile([65, QH, 512], FP16, name="on65", tag="on65",
                               bufs=2)
                nc.vector.tensor_copy(
                    on65[:, :, 0:256].rearrange("P (a i) t -> P a i t", i=2),
                    outp[:, :, 0, :, :])
                nc.scalar.activation(
                    on65[:, :, 256:512].rearrange("P (a i) t -> P a i t",
                                                  i=2),
                    outp[:, :, 1, :, :], AF.Copy)
                if KDBG == "att":
                    if qb == 0:
                        ytd = pC.tile([65, QH * 512], F32, name="ytd")
                        nc.vector.tensor_copy(
                            ytd, on65.rearrange("P h t -> P (h t)"))
                        nc.sync.dma_start(out=t["y"][0:65, :], in_=ytd)
                        ytd2 = pC.tile([128, NKT * 65], F32, name="ytd2")
                        nc.vector.tensor_copy(
                            ytd2, vtok.rearrange("p k c -> p (k c)"))
                        nc.sync.dma_start(
                            out=t["y"][128:256, 0:NKT * 65], in_=ytd2)
                    continue
                cci = t["cc_in"][qb // 2]
                for j in range(4):
                    d = 4 * (qb % 2) + j
                    tj = slice(128 * j, 128 * j + 128)
                    nc.gpsimd.dma_start(
                        out=cci[d, 0:QF, :].rearrange("(h p) t -> p h t",
                                                      p=64),
                        in_=on65[0:64, :, tj])
                    for h in range(QH):
                        nc.gpsimd.dma_start(
                            out=cci[d, QF + h:QF + h + 1, :],
                            in_=on65[64:65, h, tj])
                if qb % 2 == 1 and KDBG != "att":
                    i = qb // 2
                    nc.gpsimd.collective_compute(
                        "AllToAll", AluOpType.bypass,
                        ins=[t["cc_in"][i][:]],
                        outs=[t["cc_out"][i][:]],
                        replica_groups=[list(range(NCORES))],
                    )

        if KDBG == "att":
            return
        # ================= Phase D =================
        with nc.named_scope("phaseD"), \
                tc.tile_pool(name="pD", bufs=1) as pD, \
                tc.tile_pool(name="psD", bufs=1, space="PSUM") as ps:
            for i in range(2):
                cco = t["cc_out"][i]
                g = pD.tile([128, NIF, TD], FP16, name="g", tag="g%d" % i)
                gv = g.rearrange("p (c j) t -> p c j t", j=2)
                for j in range(2):
                    for ch in range(2):
                        ENGS[(2 * j + ch) % 3].dma_start(
                            out=gv[:, 4 * ch:4 * ch + 4, j, :],
                            in_=cco[4 * ch:4 * ch + 4,
                                    128 * j:128 * j + 128, :]
                            .rearrange("c p t -> p c t"))
                den = pD.tile([32, TD], FP16, name="den", tag="den", bufs=2)
                for cb in range(NCORES):
                    nc.sync.dma_start(
                        out=den[QH * cb:QH * cb + QH, :],
                        in_=cco[cb, QF:QF + QH, :])
                if KDBG == "gd":
                    gtd = pD.tile([128, NIF * TD], F32, name="gtd",
                                  tag="gtd", bufs=2)
                    nc.vector.tensor_copy(
                        gtd, g.rearrange("p k t -> p (k t)"))
                    if i == 0:
                        nc.sync.dma_start(out=t["y"][0:128, :], in_=gtd)
                    dtd = pD.tile([32, TD], F32, name="dtd", tag="dtd",
                                  bufs=2)
                    nc.vector.tensor_copy(dtd, den)
                    nc.sync.dma_start(
                        out=t["y"][128 + 32 * i:160 + 32 * i, 0:TD], in_=dtd)
                    continue
                rec = pD.tile([32, TD], FP16, name="rec", tag="rec", bufs=2)
                with nc.allow_low_precision(reason="attn denom recip"):
                    nc.vector.reciprocal(rec, den)
                # software-pipelined: normalize chunk k, then immediately its
                # Y and o-lora-A matmuls (Y streams while DVE normalizes k+1)
                Y = ps.tile([128, 4, 512], F32, name="Y", tag="b_Y")
                ho = ps.tile([128, TD], F32, name="ho", tag="b_ho")
                for k in range(NIF):
                    rb = ps.tile([128, TD], F32, name="rb",
                                 tag="b_rb%d" % (k % 2))
                    nc.tensor.matmul(rb, selk_sb[:, k, :], rec,
                                     start=True, stop=True)
                    nc.vector.tensor_tensor(g[:, k, :], g[:, k, :], rb,
                                            AluOpType.mult)
                    for ob in range(4):
                        nc.tensor.matmul(
                            Y[:, ob, :], g[:, k, :],
                            woT_sb[:, k, 512 * ob:512 * ob + 512],
                            start=(k == 0), stop=False)
                    nc.tensor.matmul(ho[0:72, :], aob_sb[:, k, :], g[:, k, :],
                                     start=(k == 0), stop=(k == NIF - 1))
                ex_o = pD.tile([8, TD], FP16, name="ex_o", tag="ex_o", bufs=2)
                nc.scalar.activation(ex_o, ho[64:72, :], AF.Exp)
                sm_o = ps.tile([128, TD], F32, name="sm_o", tag="b_rb0")
                nc.tensor.matmul(sm_o[0:8, :], ones8x8, ex_o,
                                 start=True, stop=True)
                rec_o = pD.tile([8, TD], FP16, name="rec_o", tag="rec_o",
                                bufs=2)
                with nc.allow_low_precision(reason="o router recip"):
                    nc.vector.reciprocal(rec_o, sm_o[0:8, :])
                rw_o = pD.tile([8, TD], FP16, name="rw_o", tag="rw_o", bufs=2)
                nc.vector.tensor_tensor(rw_o, ex_o, rec_o, AluOpType.mult)
                rt = ps.tile([64, TD], F32, name="rt", tag="b_rt")
                nc.tensor.matmul(rt, sel_o, rw_o, start=True, stop=True)
                ho_sb = pD.tile([64, TD], FP16, name="ho_sb", tag="ho_sb",
                                bufs=2)
                nc.vector.tensor_copy(ho_sb, ho[0:64, :])
                hpo = pD.tile([64, TD], FP16, name="hpo", tag="hpo", bufs=2)
                nc.vector.tensor_tensor(hpo, ho_sb, rt, AluOpType.mult)

                for ob in range(4):
                    nc.tensor.matmul(Y[:, ob, :], hpo,
                                     bo_sb[:, 512 * ob:512 * ob + 512],
                                     start=False, stop=True)
                yt = pD.tile([128, 4, 512], F32, name="yt", tag="yt", bufs=2)
                nc.vector.tensor_copy(yt[:, 0, :], Y[:, 0, :])
                nc.scalar.activation(yt[:, 1, :], Y[:, 1, :], AF.Copy)
                nc.vector.tensor_copy(yt[:, 2, :], Y[:, 2, :])
                nc.scalar.activation(yt[:, 3, :], Y[:, 3, :], AF.Copy)
                nc.sync.dma_start(
                    out=t["y"][TD * i:TD * i + TD, 0:1024],
                    in_=yt[:, 0:2, :].rearrange("p a f -> p (a f)"))
                nc.scalar.dma_start(
                    out=t["y"][TD * i:TD * i + TD, 1024:2048],
                    in_=yt[:, 2:4, :].rearrange("p a f -> p (a f)"))


# ======================= host side =======================

_CACHE = {}


def _prep_inputs(x, mask, freqs_cos, freqs_sin, wq, wk, wv, wo,
                 lq_router, lq_A, lq_B, lk_router, lk_A, lk_B,
                 lv_router, lv_A, lv_B, lo_router, lo_A, lo_B):
    scale = 1.0 / np.sqrt(HD)
    x = _f32(np.asarray(x)).reshape(S, D)
    maskf = _f32(np.asarray(mask)).reshape(S, S)
    maskT = np.maximum(maskf, MASK_NEG).T.copy()
    cls, patterns = classify_mask(maskT)
    n_pat = len(patterns)
    if n_pat:
        masku = np.stack(patterns, axis=1).reshape(128, n_pat * 256)
    else:
        masku = np.zeros((128, 256), dtype=np.float32)

    cos = _f32(freqs_cos)  # [S, 32]
    sin = _f32(freqs_sin)
    cs4 = _fp16(np.tile(cos.T, (4, 1)))
    sn4 = _fp16(np.tile(sin.T, (4, 1)))

    # selector pack [24, 516] (cols: sel_qk 0:128 | sel_v 128:192 |
    #   ones_blk 192:216 | sel_o 216:280 | ones8x8 280:288)
    sels = np.zeros((24, 516), dtype=np.float32)
    for e in range(E):
        for r in range(R):
            sels[e, r * 8 + e] = 1.0                  # sel_qk (q)
            sels[8 + e, 64 + r * 8 + e] = 1.0         # sel_qk (k)
            sels[16 + e, 128 + r * 8 + e] = 1.0       # sel_v
            sels[e, 216 + r * 8 + e] = 1.0            # sel_o
    for j in range(24):
        sels[j, 192 + (j // 8) * 8:192 + (j // 8) * 8 + 8] = 1.0  # ones_blk
    sels[0:8, 280:288] = 1.0                          # ones8x8

    selkm = np.zeros((32, NIF * 128), dtype=np.float32)
    for k in range(NIF):
        for p in range(128):
            selkm[2 * k + p // 64, 128 * k + p] = 1.0

    ao_p = np.concatenate([_a_pack(_f32(lo_A)), _f32(lo_router).T], axis=1)
    shared = dict(xT=_fp16(x.T), cs4=cs4, sn4=sn4,
                  woT=_fp16(_sw(_f32(wo).T)),
                  masku=_fp16(masku), sels=_fp16(sels), selk=_fp16(selkm),
                  aob=_fp16(_sw(ao_p)),
                  bo=_fp16(_b_flat(_f32(lo_B), SCALING)))

    aq_p = _a_pack(_f32(lq_A))
    ak_p = _a_pack(_f32(lk_A))
    av_p = _a_pack(_f32(lv_A))
    aqk_p = _fp16(_sw(np.concatenate([aq_p, ak_p], axis=1)))
    avr_p = _fp16(_sw(np.concatenate(
        [av_p, _f32(lq_router).T, _f32(lk_router).T, _f32(lv_router).T],
        axis=1)))

    wqf, wkf, wvf = _f32(wq), _f32(wk), _f32(wv)
    lqB, lkB, lvB = _f32(lq_B), _f32(lk_B), _f32(lv_B)

    in_maps = []
    for c in range(NCORES):
        wq_c = wqf[c * QF:(c + 1) * QF][IDX_Q] * scale
        wk_c = wkf[c * HD:(c + 1) * HD][IDX_K]
        wv_c = wvf[c * HD:(c + 1) * HD]
        wkv_c = np.concatenate([wk_c, wv_c], axis=0)
        bq_c = _b_flat(lqB[:, c * QF:(c + 1) * QF, :][:, IDX_Q, :],
                       SCALING * scale)
        bk_c = _b_flat(lkB[:, c * HD:(c + 1) * HD, :][:, IDX_K, :], SCALING)
        bkv_c = np.zeros((128, 64), dtype=np.float32)
        bkv_c[64:128] = bk_c
        bv_c = _b_flat(lvB[:, c * HD:(c + 1) * HD, :], SCALING)
        m = dict(shared)
        m.update(wqT=_fp16(_sw(wq_c.T)), wkvT=_fp16(_sw(wkv_c.T)),
                 aqk=aqk_p, avr=avr_p,
                 bq=_fp16(bq_c), bkv=_fp16(bkv_c), bv=_fp16(bv_c))
        in_maps.append(m)
    return in_maps, cls, n_pat


def get_graph(cls, n_pat):
    key = (cls.tobytes(), n_pat, KDBG)
    if key not in _CACHE:
        _CACHE[key] = build(cls, n_pat)
    return _CACHE[key]


def kernel(x, start_pos, mask, freqs_cos, freqs_sin, wq, wk, wv, wo,
           lq_router, lq_A, lq_B, lk_router, lk_A, lk_B,
           lv_router, lv_A, lv_B, lo_router, lo_A, lo_B,
           _trace=False):
    from concourse.bass_utils import run_bass_kernel_spmd
    in_maps, cls, n_pat = _prep_inputs(
        x, mask, freqs_cos, freqs_sin, wq, wk, wv, wo,
        lq_router, lq_A, lq_B, lk_router, lk_A, lk_B,
        lv_router, lv_A, lv_B, lo_router, lo_A, lo_B)
    nc = get_graph(cls, n_pat)
    res = run_bass_kernel_spmd(nc, in_maps, list(range(NCORES)), trace=_trace)
    out = np.empty((S, D), dtype=np.float32)
    for c in range(NCORES):
        yc = res.results[c]["y"]
        out[TD * c:TD * c + TD] = yc[0:TD]
        out[S // 2 + TD * c:S // 2 + TD * c + TD] = yc[TD:2 * TD]
    out = out.reshape(B, S, H * HD)
    if _trace:
        return out, res
    return out
